# revision 4
# baseline (speedup 1.0000x reference)
"""Block-sparse matmul + bias + relu on 8 Trainium2 NeuronCores.

Strategy (data-parallel over batch):
  - Shard x along batch: 8 cores x 512 rows. w_blocks/bias replicated.
  - Per core, out^T computed with the PE in 32x32 tiling mode:
      * x^T resident in SBUF as [128, 32, 512]: input block i lives at
        partitions 32*(i%4) .. 32*(i%4)+31, free tile i//4.
      * each nonzero block (i,j) is one matmul: lhsT = w block [K=32, M=32],
        rhs = x^T block i [32, 512], accumulated at partition strip
        32*(j%4) of the row-group bank: tile_position=(32*(i%4), 32*(j%4)).
      * output block-cols processed in quads (4 cols -> 4 strips x 4 banks).
  - Per-matmul semaphore increments are stripped post-Tile (only increments
    some wait actually references are kept; matmuls complete in pc order so
    rank-remapped thresholds are equivalent) - saves ~26 ns/matmul of
    serialized EVT_SEM writes.
  - Per quad combine (engine-balanced, one PSUM operand per DVE op):
      ACT: e0 = Identity(acc0 + bias_q), e2 = Identity(acc2)
      DVE: s1 = (acc1 + 0) + e0,  s2 = (acc3 + 0) + e2   (STT)
      GPS: s3 = s1 + s2
      DVE: ot = max(s3, 0) -> bf16, DMA out^T tile to DRAM.
  - Output returned as bf16 from device, upcast to fp32 on host.
"""

import os

import numpy as np
import ml_dtypes

import concourse.bass as bass
import concourse.tile as tile
from concourse import mybir
from concourse.bass_utils import run_bass_kernel_spmd

LAST_RESULTS = None  # test-only: BassKernelResults of the last run

BS = 32
KB = 128
NB = 128
BATCH = 4096
NCORES = 8
BC = BATCH // NCORES          # 512 batch rows per core
NQ = NB // 4                  # 32 quads of output block-cols
IN_DT = mybir.dt.bfloat16
IN_NP = ml_dtypes.bfloat16
OUT_DT = mybir.dt.bfloat16
OUT_NP = ml_dtypes.bfloat16

_CACHE = {}


def _build_schedule(row_idx, col_idx):
    """Strip-major emission: per quad, row-group r outer; within a strip,
    round-robin the 4 col FIFOs (4-way col-tile concurrency) with weight
    slots assigned in emission order, so consecutive LDWEIGHTS read
    sequential SBUF addresses (measured ~2x faster than interleaving row
    groups). Returns (sched, S, slot_of, dummy_slots): sched[q] lists
    (r, c, t, slot, start, stop); S is the per-strip slot count."""
    nnz = len(row_idx)
    fifos = [[[[] for _ in range(4)] for _ in range(4)] for _ in range(NQ)]
    for n in range(nnz):
        i = int(row_idx[n]); j = int(col_idx[n])
        fifos[j // 4][j % 4][i % 4].append(n)

    slot_ctr = [0, 0, 0, 0]           # per row-group strip
    sched = []
    slot_of = {}                      # block id -> slot (in its strip)
    dummy_slots = []                  # (r, slot) zero-weight slots
    for q in range(NQ):
        for c in range(4):
            for r in range(4):
                if not fifos[q][c][r]:
                    fifos[q][c][r].append(None)
        emitted = []
        for r in range(4):
            maxlen = max(len(fifos[q][c][r]) for c in range(4))
            for s in range(maxlen):
                for c in range(4):
                    lst = fifos[q][c][r]
                    if s < len(lst):
                        n = lst[s]
                        slot = slot_ctr[r]; slot_ctr[r] += 1
                        if n is None:
                            dummy_slots.append((r, slot))
                            t = 0
                        else:
                            slot_of[n] = slot
                            t = int(row_idx[n]) // 4
                        emitted.append([r, c, t, slot, False, False])
        first_seen = set()
        for e in emitted:
            key = (e[0], e[1])
            if key not in first_seen:
                e[4] = True
                first_seen.add(key)
        last_idx = {}
        for k, e in enumerate(emitted):
            last_idx[(e[0], e[1])] = k
        for k in last_idx.values():
            emitted[k][5] = True
        sched.append([tuple(e) for e in emitted])
    S = max(slot_ctr)
    return sched, S, slot_of, dummy_slots


_MULTIWAIT_OK = {"InstDMACopy", "InstUnconditionalBranch",
                 "InstConditionalBranch"}


def _legalize_waits(nc):
    """Engine ISA structs carry a single sync-wait slot; Tile can emit more.
    Offload excess waits onto same-engine NoOps inserted just before the
    instruction (per-engine stream order is the block list order)."""
    ctr = 0
    for f in nc.m.functions:
        for blk in f.blocks:
            out = []
            for inst in blk.instructions:
                si = inst.sync_info
                if (si is not None and si.on_wait and len(si.on_wait) > 1
                        and type(inst).__name__ == "InstDMACopy"):
                    # HWDGE lane sems are monotonic add-only counters; a
                    # DMA's wait on its own completion lane orders it against
                    # unrelated prior DMAs on that lane and is droppable.
                    own = {u.ant_name for u in (si.on_update or [])}
                    keep = [w for w in si.on_wait if w.ant_name not in own]
                    if len(keep) > 1:
                        raise RuntimeError(
                            f"DMA {inst.name} still has waits {keep}")
                    inst.sync_info = mybir.SyncInfo(on_wait=keep,
                                                    on_update=si.on_update)
                    out.append(inst)
                    continue
                if (si is not None and si.on_wait and len(si.on_wait) > 1
                        and type(inst).__name__ not in _MULTIWAIT_OK):
                    waits = list(si.on_wait)
                    for w in waits[:-1]:
                        nop = mybir.InstNoOp(name=f"waitnop-{ctr}")
                        ctr += 1
                        nop.engine = inst.engine
                        nop.sync_info = mybir.SyncInfo(on_wait=[w], on_update=[])
                        out.append(nop)
                    inst.sync_info = mybir.SyncInfo(on_wait=[waits[-1]],
                                                    on_update=si.on_update)
                out.append(inst)
            blk.instructions[:] = out


def _strip_mm_sem_incs(nc):
    """Drop per-matmul semaphore increments except those whose count some
    wait references; remap wait thresholds and the loop pre-credit/debit
    amounts to the compacted count. Matmuls complete in pc order, so waiting
    for the rank(v)-th kept increment is equivalent to the original v-th."""
    mm_sems = set()
    for f in nc.m.functions:
        for blk in f.blocks:
            for inst in blk.instructions:
                if (getattr(inst, 'engine', None) == mybir.EngineType.PE
                        and type(inst).__name__ == "InstMatmult"):
                    si = inst.sync_info
                    if si is None:
                        continue
                    for u in (si.on_update or []):
                        if u.update_mode == "sem-inc" and u.update_value == 1:
                            mm_sems.add(u.ant_name)
    for sem in mm_sems:
        ok = True
        total = 0
        thresholds = set()
        adjusts = []
        mm_counts = {}
        for f in nc.m.functions:
            for blk in f.blocks:
                c = 0
                for inst in blk.instructions:
                    si = inst.sync_info
                    if si is None:
                        continue
                    for u in (si.on_update or []):
                        if u.ant_name != sem:
                            continue
                        if u.update_mode == "sem-inc" and u.update_value == 1:
                            if type(inst).__name__ != "InstMatmult":
                                ok = False
                            c += 1
                            mm_counts[id(inst)] = c
                        elif u.update_mode in ("sem-add-imm", "sem-sub-imm"):
                            adjusts.append(u)
                        else:
                            ok = False
                    for w in (si.on_wait or []):
                        if w.ant_name == sem and w.wait_mode != "sem-ge-imm":
                            ok = False
                total = max(total, c)
        for f in nc.m.functions:
            for blk in f.blocks:
                for inst in blk.instructions:
                    si = inst.sync_info
                    if si is None:
                        continue
                    for w in (si.on_wait or []):
                        if w.ant_name == sem and w.wait_value > 0:
                            thresholds.add(w.wait_value)
        if not ok or total == 0:
            continue
        if any(u.update_value != total for u in adjusts):
            continue
        keep = sorted(thresholds | {total})
        rank = {v: i + 1 for i, v in enumerate(keep)}
        newtotal = len(keep)
        for f in nc.m.functions:
            for blk in f.blocks:
                for inst in blk.instructions:
                    si = inst.sync_info
                    if si is None:
                        continue
                    changed = False
                    new_waits = []
                    for w in (si.on_wait or []):
                        if w.ant_name == sem and w.wait_value > 0:
                            new_waits.append(mybir.SyncWait(
                                sync_type=w.sync_type, id=w.id,
                                ant_name=w.ant_name, wait_mode=w.wait_mode,
                                wait_value=rank[w.wait_value],
                                wait_reg=w.wait_reg))
                            changed = True
                        else:
                            new_waits.append(w)
                    new_upds = []
                    for u in (si.on_update or []):
                        if u.ant_name != sem:
                            new_upds.append(u)
                            continue
                        if u.update_mode == "sem-inc":
                            if mm_counts[id(inst)] in rank:
                                new_upds.append(u)
                            changed = True
                        elif u.update_mode in ("sem-add-imm", "sem-sub-imm"):
                            new_upds.append(mybir.SyncUpdate(
                                sync_type=u.sync_type, id=u.id,
                                ant_name=u.ant_name,
                                update_mode=u.update_mode,
                                update_value=newtotal,
                                update_reg=u.update_reg))
                            changed = True
                        else:
                            new_upds.append(u)
                    if changed:
                        inst.sync_info = mybir.SyncInfo(on_wait=new_waits,
                                                        on_update=new_upds)
    return nc


def _build_program(sched, S, repeat=1, loop_n=0):
    import contextlib
    nc = bass.Bass("TRN2", target_bir_lowering=False, debug=False,
                   num_devices=NCORES)
    x_d = nc.dram_tensor("xt", [128, 32 * BC], IN_DT, kind="ExternalInput").ap()
    w_d = nc.dram_tensor("wim", [128, S * 32], IN_DT, kind="ExternalInput").ap()
    b_d = nc.dram_tensor("bias", [128, 32], mybir.dt.float32,
                         kind="ExternalInput").ap()
    o_d = nc.dram_tensor("outT", [NQ, 128, BC], OUT_DT, kind="ExternalOutput").ap()

    with tile.TileContext(nc) as tc:
        loop_cm = tc.For_i(0, loop_n, 1) if loop_n else contextlib.nullcontext()
        with tc.tile_pool(name="const", bufs=2) as cpool, \
             tc.tile_pool(name="work", bufs=3) as wpool, \
             tc.tile_pool(name="psum", bufs=2, space="PSUM") as ppool, \
             loop_cm:
            xt = cpool.tile([128, 32 * BC], IN_DT)
            wt = cpool.tile([128, S * 32], IN_DT)
            bt = cpool.tile([128, 32], mybir.dt.float32)
            nc.sync.dma_start(bt[:], b_d[:])
            # x: chunked DMA (16 x 1MB)
            xch = (32 * BC) // 16
            for k in range(16):
                nc.sync.dma_start(xt[:, k * xch:(k + 1) * xch],
                                  x_d[:, k * xch:(k + 1) * xch])
            # w: chunked DMA in slot order so early quads unblock early
            wstep = -(-S // 8) * 32
            for k in range(8):
                lo = k * wstep
                hi = min(S * 32, lo + wstep)
                if lo >= hi:
                    continue
                nc.sync.dma_start(wt[:, lo:hi], w_d[:, lo:hi])

            for rep in range(repeat):
              for q in range(NQ):
                acc = [ppool.tile([128, BC], mybir.dt.float32, tag=f"acc{r}",
                                  name=f"acc{r}_q{q}_p{rep}")
                       for r in range(4)]
                for (r, c, t, slot, start, stop) in sched[q]:
                    nc.tensor.matmul(
                        out=acc[r][32 * c:32 * c + 32, :],
                        lhsT=wt[32 * r:32 * r + 32,
                                slot * 32:(slot + 1) * 32],
                        rhs=xt[32 * r:32 * r + 32, t * BC:(t + 1) * BC],
                        start=start, stop=stop,
                        tile_position=(32 * r, 32 * c),
                        skip_group_check=True,
                    )
                e0 = wpool.tile([128, BC], mybir.dt.float32, tag="e0")
                e2 = wpool.tile([128, BC], mybir.dt.float32, tag="e2")
                s1 = wpool.tile([128, BC], mybir.dt.float32, tag="s1")
                s2 = wpool.tile([128, BC], mybir.dt.float32, tag="s2")
                s3 = wpool.tile([128, BC], mybir.dt.float32, tag="s3")
                ot = wpool.tile([128, BC], OUT_DT, tag="ot")
                nc.scalar.activation(e0[:], acc[0][:],
                                     mybir.ActivationFunctionType.Identity,
                                     bias=bt[:, q:q + 1], scale=1.0)
                nc.scalar.activation(e2[:], acc[2][:],
                                     mybir.ActivationFunctionType.Identity,
                                     bias=0.0, scale=1.0)
                nc.vector.scalar_tensor_tensor(
                    s1[:], acc[1][:], 0.0, e0[:],
                    mybir.AluOpType.add, mybir.AluOpType.add)
                nc.vector.scalar_tensor_tensor(
                    s2[:], acc[3][:], 0.0, e2[:],
                    mybir.AluOpType.add, mybir.AluOpType.add)
                nc.gpsimd.tensor_add(s3[:], s1[:], s2[:])
                nc.vector.tensor_scalar_max(ot[:], s3[:], 0.0)
                nc.sync.dma_start(o_d[q], ot[:])
    _legalize_waits(nc)
    _strip_mm_sem_incs(nc)
    return nc


def _prep_inputs(x, w_blocks, bias, row_idx, col_idx, slot_of, dummy_slots, S):
    nnz = len(row_idx)
    # x^T images per core: block i at partitions 32*(i%4), free tile i//4.
    xb = x.astype(IN_NP).reshape(BATCH, 32, 4, 32)        # b, t, r, p
    xt_all = np.ascontiguousarray(xb.transpose(2, 3, 1, 0))  # r, p, t, b
    xt_all = xt_all.reshape(128, 32, BATCH)
    xts = [np.ascontiguousarray(xt_all[:, :, c * BC:(c + 1) * BC]
                                ).reshape(128, 32 * BC) for c in range(NCORES)]
    # w image [128, S*32]
    wim = np.zeros((128, S * 32), dtype=IN_NP)
    wb = w_blocks.astype(IN_NP)
    for n in range(nnz):
        r = int(row_idx[n]) % 4
        s = slot_of[n]
        wim[32 * r:32 * r + 32, 32 * s:32 * s + 32] = wb[n]
    bim = np.ascontiguousarray(
        bias.astype(np.float32).reshape(32, 4, 32).transpose(1, 2, 0)
    ).reshape(128, 32)
    return xts, wim, bim


def kernel(x, w_blocks, bias, row_idx, col_idx):
    key = (row_idx.tobytes(), col_idx.tobytes())
    if key not in _CACHE:
        sched, S, slot_of, dummy_slots = _build_schedule(row_idx, col_idx)
        nc = _build_program(sched, S)
        _CACHE[key] = (nc, S, (slot_of, dummy_slots))
    nc, S, aux = _CACHE[key]
    slot_of, dummy_slots = aux

    xts, wim, bim = _prep_inputs(x, w_blocks, bias, row_idx, col_idx,
                                 slot_of, dummy_slots, S)
    in_maps = [{"xt": xts[c], "wim": wim, "bias": bim} for c in range(NCORES)]
    res = run_bass_kernel_spmd(nc, in_maps, list(range(NCORES)))
    global LAST_RESULTS
    LAST_RESULTS = res

    out = np.empty((BATCH, NB * BS), dtype=np.float32)
    for c in range(NCORES):
        outT = res.results[c]["outT"].reshape(NB * BS, BC)
        out[c * BC:(c + 1) * BC, :] = outT.T.astype(np.float32)
    return out
